# revision 38
# baseline (speedup 1.0000x reference)
"""Trainium2 Bass kernel for linear causal self-attention (ELU+1 feature map).

Model (per batch b):
    qkv = x @ W_attn + b_attn ; q,k,v split; per-head (H=16, d=64)
    phi = elu(.)+1 applied to q,k
    causal linear attention: y_t = (phi_q_t . KV_t) / (phi_q_t . Ksum_t + eps)
        KV_t = sum_{s<=t} phi_k_s (x) v_s ; Ksum_t = sum_{s<=t} phi_k_s
    out = y @ W_proj + b_proj

Sharding (8 cores): core = 2*b + g  (b in 0..3 batches, g in 0..1 head-groups
of 8 heads).  Each core computes a partial output for its batch over its 8
heads; the host sums the two partials per batch and adds b_proj.

Performance design (from HW traces of earlier revisions):
  * The PE clock is HAM-throttled to 1.2 GHz whenever a ~3.4us activity
    window looks idle; small attention matmuls alone (65-128 cols) cannot
    keep it warm.  So the big 512-col qkv chains for slab s+1 and the
    output-projection chains for slab s-2 are emitted as *fine-grained
    fillers* - a few matmuls after every attention chunk unit - keeping the
    array duty-cycle high everywhere, including the last slab.
  * Attention chunks are S=128: no wasted half-block in the causal scores.
  * Per-head PSUM banks for scores/y: two accumulation groups from
    different PE row groups into one bank is a hard HW fault (bisected).
  * v is produced token-major directly (lhsT = x^T tile); y is normalized
    then PE-transposed per head-pair in ONE [128,128] transpose.
  * Host pre-lays-out all inputs so every DMA is 128 x 8KB descriptors;
    wq/xt0 are split across the two HWDGE rings so the first matmul can
    start ~14us in (3.3us init barrier + table loads are fixed runtime
    cost).  Output is written bf16 (tolerance 2e-2 >> bf16 rounding).
"""

from contextlib import ExitStack

import ml_dtypes
import numpy as np

import concourse.bass as bass
import concourse.mybir as mybir
import concourse.tile as tile
from concourse.bass_utils import run_bass_kernel_spmd
from concourse.masks import make_identity

F32 = mybir.dt.float32
BF16 = mybir.dt.bfloat16
AF = mybir.ActivationFunctionType

# Problem shape (hardcoded per harness contract)
B, T, C = 4, 2048, 1024
NH, D = 16, 64          # heads total, head dim
HG = 8                  # heads per core (head-group)
HP = 4                  # head-pairs per core (2 heads stacked on 128 partitions)
FC = HG * D             # 512 features per core per q/k/v
KT = C // 128           # 8 k-tiles of the model dim
SLAB = 512              # tokens per slab (qkv/proj granularity)
NSLAB = T // SLAB       # 4
NC_ = 4                 # chunks (128-token tiles) per slab
S128 = 128              # chunk length
N_CORES = 8


def _split_waits(nc, max_waits=1):
    """This walrus build accepts at most one embedded sync-wait per
    instruction; hoist extras into standalone EventSemaphore instructions."""
    n = 0
    for fn in nc.m.functions:
        for bb in fn.blocks:
            new_insts = []
            for inst in bb.instructions:
                si = inst.sync_info
                if si is not None and si.on_wait and len(si.on_wait) > max_waits:
                    extra = si.on_wait[:-max_waits]
                    keep = si.on_wait[-max_waits:]
                    for w in extra:
                        ev = mybir.InstEventSemaphore(
                            name=f"{inst.name}-wsplit{n}",
                            ins=[], outs=[],
                            engine=inst.engine,
                            sync_info=mybir.SyncInfo(on_wait=[w], on_update=[]),
                        )
                        n += 1
                        new_insts.append(ev)
                    si.on_wait = list(keep)
                new_insts.append(inst)
            bb.instructions = new_insts
    return n


class _Ctx:
    """Holds nc, pools and per-kernel constant tiles."""


def _setup_consts(g: _Ctx, dram):
    nc, consts = g.nc, g.consts
    # DMA ordering matters: each HWDGE ring completes in FIFO order.  The
    # first q-chain needs wq + xt0, so each is split half per ring and
    # leads its ring; everything else streams behind.
    g.wq_sb = consts.tile([128, KT, FC], BF16, tag="wq", name="wq")
    g.xt = []
    for s in range(NSLAB):
        g.xt.append(consts.tile([128, KT, SLAB], BF16, tag=f"xt{s}", name=f"xt{s}"))
    nc.sync.dma_start(g.wq_sb[:, 0 : KT // 2], dram["wq"][:, 0 : KT // 2])
    nc.scalar.dma_start(g.xt[0][:, 0 : KT // 2], dram["xt"][:, 0, 0 : KT // 2])
    nc.sync.dma_start(g.xt[0][:, KT // 2 :], dram["xt"][:, 0, KT // 2 :])
    nc.scalar.dma_start(g.wq_sb[:, KT // 2 :], dram["wq"][:, KT // 2 :])
    # per-partition phi biases: cols [bq, bq1, bk, bk1] x HP
    g.bias_sb = consts.tile([128, 4, HP], F32, tag="bias", name="bias")
    nc.scalar.dma_start(g.bias_sb, dram["bias"][:])
    g.wk_sb = consts.tile([128, KT, FC], BF16, tag="wk", name="wk")
    nc.sync.dma_start(g.wk_sb[:, 0 : KT // 2], dram["wk"][:, 0 : KT // 2])
    nc.scalar.dma_start(g.wk_sb[:, KT // 2 :], dram["wk"][:, KT // 2 :])
    nc.scalar.dma_start(g.xt[1], dram["xt"][:, 1])
    g.wv_sb = consts.tile([128, KT, FC], BF16, tag="wv", name="wv")
    nc.sync.dma_start(g.wv_sb, dram["wv"][:])
    # v-bias broadcast tile [128, FC]: every partition row = b_v
    g.bvb = consts.tile([128, FC], F32, tag="bvb", name="bvb")
    nc.scalar.dma_start(g.bvb, dram["bvb"][:])
    g.wp_sb = consts.tile([128, HP, C], BF16, tag="wp", name="wp")
    nc.sync.dma_start(g.wp_sb, dram["wp"][:])
    for s in range(2, NSLAB):
        nc.sync.dma_start(g.xt[s], dram["xt"][:, s])

    # causal mask for a [s_k(128), head(2), s_q(128)] score tile: keep
    # s_q >= s_k (same triangle in both head slots)
    g.mask = consts.tile([128, 2, S128], F32, tag="mask", name="mask")
    nc.vector.memset(g.mask, 1.0)
    for h in range(2):
        nc.gpsimd.affine_select(
            out=g.mask[:, h], in_=g.mask[:, h],
            compare_op=mybir.AluOpType.is_ge,
            fill=0.0, base=0, pattern=[[1, S128]], channel_multiplier=-1,
        )
    g.ident = consts.tile([128, 128], BF16, tag="ident", name="ident")
    make_identity(nc, g.ident)

    # combined state master (f32), all head-pairs: rows = 2 heads x 64,
    # per hp cols 0:64 = KV[i, j], col 64 = Ksum[i]
    g.kv = g.statep.tile([128, HP, D + 1], F32, tag="kv", name="kv")
    nc.vector.memset(g.kv, 0.0)


def _q_or_k_unit(g: _Ctx, s, hp, which):
    """One feature-major q/k chain + phi for slab s, head-pair hp.
    phi(y) = elu(y)+1 = max(min(exp(y), 1), y+1), computed as
    z = y+1 (ACT, bias), e = exp(y) (ACT, bias), e = min(e,1) (DVE),
    dst = max(z, e) (DVE).  Generator: yields after each PE instruction."""
    nc = g.nc
    wsb = g.wq_sb if which == "q" else g.wk_sb
    bcol = 0 if which == "q" else 2  # [bq, bq1, bk, bk1]
    ps = g.ps_big.tile([128, SLAB], F32, tag="big", name="qkps")
    xt_sb = g.xt[s]
    for k in range(KT):
        nc.tensor.matmul(
            ps,
            wsb[:, k, hp * 128 : (hp + 1) * 128],
            xt_sb[:, k, :],
            start=(k == 0),
            stop=(k == KT - 1),
        )
        if k < KT - 1:
            yield
    z = g.ep.tile([128, SLAB], BF16, tag="z", name="z")
    nc.scalar.activation(
        z, ps, AF.Identity, bias=g.bias_sb[:, bcol + 1, hp : hp + 1], scale=1.0
    )
    e = g.ep.tile([128, SLAB], BF16, tag="e", name="e")
    nc.scalar.activation(
        e, ps, AF.Exp, bias=g.bias_sb[:, bcol, hp : hp + 1], scale=1.0
    )
    nc.vector.tensor_scalar_min(e, e, 1.0)
    dst = g.qkp.tile([128, SLAB], BF16, tag=f"{which}t{hp}", name=f"{which}t{hp}")
    nc.vector.tensor_max(dst, z, e)
    lst = g.qt if which == "q" else g.kt_
    lst[(s % 2) * HP + hp] = dst
    yield


def _v_unit(g: _Ctx, s, tt):
    """Token-major v for slab s, token-tile tt: direct matmul with
    lhsT = x^T tile, plus broadcast bias add and the ones column."""
    nc = g.nc
    ps = g.ps_big.tile([128, SLAB], F32, tag="big", name="vps")
    xt_sb = g.xt[s]
    for k in range(KT):
        nc.tensor.matmul(
            ps[:, 0:FC],
            xt_sb[:, k, tt * 128 : (tt + 1) * 128],
            g.wv_sb[:, k, :],
            start=(k == 0),
            stop=(k == KT - 1),
        )
        if k < KT - 1:
            yield
    va = g.kvp.tile([128, HG, D + 1], BF16, tag=f"va{tt}", name=f"va{tt}")
    nc.vector.tensor_tensor(
        va[:, :, 0:D],
        ps[:, 0:FC].rearrange("p (a b) -> p a b", a=HG),
        g.bvb[:].rearrange("p (a b) -> p a b", a=HG),
        mybir.AluOpType.add,
    )
    nc.vector.memset(va[:, :, D], 1.0)
    g.vaug[(s % 2) * NC_ + tt] = va
    yield


def _tks_unit(g: _Ctx, s, tt):
    """PE-transpose phi(k) for token-tile tt into token-major kn."""
    nc = g.nc
    tks = g.ps_big.tile([128, HP, 128], BF16, tag="big", name="tks")
    for hp in range(HP):
        nc.tensor.transpose(
            tks[:, hp], g.kt_[(s % 2) * HP + hp][:, tt * 128 : (tt + 1) * 128],
            g.ident,
        )
        if hp < HP - 1:
            yield
    kn = g.kvp.tile([128, FC], BF16, tag=f"kn{tt}", name=f"kn{tt}")
    nc.scalar.activation(
        kn[:].rearrange("p (a b) -> p a b", a=HP), tks, AF.Copy
    )
    g.knat[(s % 2) * NC_ + tt] = kn
    yield


def _proj_unit(g: _Ctx, s, tt, out, yts_tiles):
    """Output projection for token-tile tt of slab s: two 512-wide chains
    into one [128, 1024] bf16 store.  yts_tiles are captured at creation
    time (the g.yts slots are rebound two slabs later)."""
    nc = g.nc
    t0 = s * SLAB + tt * 128
    osb = g.outp.tile([128, 2, FC], BF16, tag="osb", name="osb")
    for ec in range(2):
        ps = g.ps_big.tile([128, SLAB], F32, tag="big", name="projps")
        for fp in range(HP):
            nc.tensor.matmul(
                ps,
                yts_tiles[fp][:, tt * 128 : (tt + 1) * 128],
                g.wp_sb[:, fp, ec * FC : (ec + 1) * FC],
                start=(fp == 0),
                stop=(fp == HP - 1),
            )
            if fp < HP - 1 or ec == 0:
                yield
        nc.scalar.activation(osb[:, ec], ps, AF.Copy)
    nc.sync.dma_start(out[t0 : t0 + 128, :], osb[:].rearrange("p a b -> p (a b)"))
    yield


class _Fillers:
    """Queue of generator units; emit(n) advances across them n PE-steps."""

    def __init__(self):
        self.gens = []
        self.steps = 0  # total yields remaining (approximate bookkeeping)

    def add(self, gen, steps):
        self.gens.append(gen)
        self.steps += steps

    def emit(self, n):
        while n > 0 and self.gens:
            try:
                next(self.gens[0])
                self.steps -= 1
                n -= 1
            except StopIteration:
                self.gens.pop(0)

    def drain(self):
        while self.gens:
            self.emit(1 << 30)


def _chunk(g: _Ctx, s, c, hp, kvb, fq):
    """One (128-token chunk, head-pair) attention unit.  Filler matmuls are
    emitted at each cross-engine wait point: the PE executes in order, so a
    chunk matmul waiting on a DVE consumer would otherwise idle the array
    (and trip the HAM throttle) even with fillers queued after the chunk."""
    nc = g.nc
    q0 = c * S128
    si = s % 2
    qth = g.qt[si * HP + hp]
    kth = g.kt_[si * HP + hp]
    kn = g.knat[si * NC_ + c]
    va = g.vaug[si * NC_ + c]
    # scores P [s_k, s_q] per head, one 64-row matmul each (own bank:
    # two different-row-group groups in one bank is a HW fault)
    phs = []
    for h in range(2):
        r0, r1 = h * D, (h + 1) * D
        ph = getattr(g, f"ps_p{h}").tile([128, S128], F32, tag=f"p{h}", name=f"p{h}")
        nc.tensor.matmul(
            ph,
            kth[r0:r1, q0 : q0 + S128],
            qth[r0:r1, q0 : q0 + S128],
            tile_position=(h * D, 0),
        )
        phs.append(ph)
    pm = g.pmp.tile([128, 2, S128], BF16, tag="pm", name="pm")
    for h in range(2):
        nc.vector.tensor_mul(pm[:, h], phs[h], g.mask[:, h])
    # state update: KV += Knat^T @ V_aug (heads col-tiled, disjoint rows);
    # independent of the scores, so it runs while the mask-mul drains
    dlt = g.ps_d.tile([128, D + 1], F32, tag="d", name="dlt")
    for h in range(2):
        hv = hp * 2 + h
        nc.tensor.matmul(
            dlt[h * D : (h + 1) * D, :],
            kn[:, hv * D : (hv + 1) * D],
            va[:, hv],
            start=True, stop=True,
            tile_position=(0, h * D),
        )
    fq.emit(2)
    # y [s_q, (u, 65)] per head: intra (P @ V_aug) + inter (q . KV_aug).
    # Each matmul is split into token-halves on different PE column groups
    # (tile_position col 0/64): the halves run in separate quadrants
    # concurrently, and the 64-col stationaries load in half the time.
    ys = []
    for h in range(2):
        r0, r1 = h * D, (h + 1) * D
        hv = hp * 2 + h
        y = getattr(g, f"ps_y{h}").tile([128, D + 1], F32, tag=f"y{h}", name=f"y{h}")
        for u in range(2):
            nc.tensor.matmul(
                y[u * 64 : (u + 1) * 64, :],
                pm[:, h, u * 64 : (u + 1) * 64],
                va[:, hv],
                start=True, stop=False,
                tile_position=(0, u * 64),
            )
            nc.tensor.matmul(
                y[u * 64 : (u + 1) * 64, :],
                qth[r0:r1, q0 + u * 64 : q0 + (u + 1) * 64],
                kvb[r0:r1, hp, :],
                start=False, stop=True,
                tile_position=(h * D, u * 64),
            )
        ys.append(y)
    # normalize y_n = y[..., 0:64] / y[..., 64]; one [128,128] PE transpose
    # lands both heads directly in yts row order (h*64+d)
    yn = g.ynp.tile([128, 2, D], BF16, tag="yn", name="yn")
    for h in range(2):
        y = ys[h]
        rec = g.ynp.tile([128, 1], F32, tag="rec", name="rec")
        nc.vector.reciprocal(rec, y[:, D : D + 1])
        nc.vector.tensor_mul(yn[:, h], y[:, 0:D], rec[:].to_broadcast((128, D)))
    fq.emit(2)
    tp = g.ps_t.tile([128, 128], BF16, tag="tp", name="tp")
    nc.tensor.transpose(tp, yn[:].rearrange("p a b -> p (a b)"), g.ident)
    nc.scalar.activation(g.yts[si * HP + hp][:, q0 : q0 + S128], tp, AF.Copy)
    # state add (scheduled after the kvb copy for this row)
    nc.vector.tensor_add(g.kv[:, hp], g.kv[:, hp], dlt)


def _new_slab_tiles(g: _Ctx, s):
    si = s % 2
    for hp in range(HP):
        g.yts[si * HP + hp] = g.ytp.tile(
            [128, SLAB], BF16, tag=f"yt{hp}", name=f"yt{hp}"
        )


def build_nc(split_waits: bool = True) -> bass.Bass:
    nc = bass.Bass()
    dram = {
        "xt": nc.dram_tensor("xt", [128, NSLAB, KT, SLAB], BF16, kind="ExternalInput"),
        "wq": nc.dram_tensor("wq", [128, KT, FC], BF16, kind="ExternalInput"),
        "wk": nc.dram_tensor("wk", [128, KT, FC], BF16, kind="ExternalInput"),
        "wv": nc.dram_tensor("wv", [128, KT, FC], BF16, kind="ExternalInput"),
        "wp": nc.dram_tensor("wp", [128, HP, C], BF16, kind="ExternalInput"),
        "bias": nc.dram_tensor("bias", [128, 4, HP], F32, kind="ExternalInput"),
        "bvb": nc.dram_tensor("bvb", [128, FC], F32, kind="ExternalInput"),
    }
    out = nc.dram_tensor("out", [T, C], BF16, kind="ExternalOutput")

    with ExitStack() as ctx:
        tc = ctx.enter_context(tile.TileContext(nc))
        g = _Ctx()
        g.nc = nc
        for nm, kw in (
            ("consts", dict(bufs=1)),
            ("qkp", dict(bufs=2)),
            ("kvp", dict(bufs=2)),
            ("ytp", dict(bufs=3)),
            ("ep", dict(bufs=4)),
            ("pmp", dict(bufs=3)),
            ("ynp", dict(bufs=4)),
            ("outp", dict(bufs=3)),
            ("statep", dict(bufs=1)),
            ("kvbp", dict(bufs=2)),
            ("ps_big", dict(bufs=2, space="PSUM")),
            ("ps_p0", dict(bufs=1, space="PSUM")),
            ("ps_p1", dict(bufs=1, space="PSUM")),
            ("ps_y0", dict(bufs=1, space="PSUM")),
            ("ps_y1", dict(bufs=1, space="PSUM")),
            ("ps_d", dict(bufs=1, space="PSUM")),
            ("ps_t", dict(bufs=1, space="PSUM")),
        ):
            setattr(g, nm, ctx.enter_context(tc.tile_pool(name=nm, **kw)))

        # slots for double-buffered per-slab tensors (slab s uses s % 2)
        g.qt = [None] * (2 * HP)
        g.kt_ = [None] * (2 * HP)
        g.knat = [None] * (2 * NC_)
        g.vaug = [None] * (2 * NC_)
        g.yts = [None] * (2 * HP)

        _setup_consts(g, dram)

        def add_qkvnat(fq, s):
            for hp in range(HP):
                fq.add(_q_or_k_unit(g, s, hp, "q"), KT)
            for hp in range(HP):
                fq.add(_q_or_k_unit(g, s, hp, "k"), KT)
            for tt in range(NC_):
                fq.add(_tks_unit(g, s, tt), HP)
            for tt in range(NC_):
                fq.add(_v_unit(g, s, tt), KT)

        # prologue: slab 0 q/k chains + first token-tile naturalize, dense.
        # tks/v for tiles 1-3 are held back as extra filler for slab 0's
        # attention (the thinnest filler zone: it has only qkv(1) otherwise).
        fq0 = _Fillers()
        for hp in range(HP):
            fq0.add(_q_or_k_unit(g, 0, hp, "q"), KT)
        for hp in range(HP):
            fq0.add(_q_or_k_unit(g, 0, hp, "k"), KT)
        fq0.add(_tks_unit(g, 0, 0), HP)
        fq0.add(_v_unit(g, 0, 0), KT)
        fq0.drain()
        held = []
        for tt in range(1, NC_):
            held.append((_tks_unit(g, 0, tt), HP))
            held.append((_v_unit(g, 0, tt), KT))

        # proj units for slab s are deferred TWO slabs: they are the PE
        # filler through slab s+2's attention (keeps the tail warm).
        pending = [[], []]
        for s in range(NSLAB):
            _new_slab_tiles(g, s)
            fq = _Fillers()
            if s == 0:
                for gen, steps in held:
                    fq.add(gen, steps)
            if s + 1 < NSLAB:
                add_qkvnat(fq, s + 1)
            for gen, steps in pending.pop(0):
                fq.add(gen, steps)
            nchunks = NC_ * HP
            for c in range(NC_):
                # held naturalize units for this row must have been emitted
                while g.knat[(s % 2) * NC_ + c] is None or (
                    g.vaug[(s % 2) * NC_ + c] is None
                ):
                    fq.emit(1)
                kvb = g.kvbp.tile([128, HP, D + 1], BF16, tag="kvb", name="kvb")
                nc.scalar.activation(kvb, g.kv, AF.Copy)
                for hp in range(HP):
                    _chunk(g, s, c, hp, kvb, fq)
                    left = NC_ * HP - (c * HP + hp)
                    fq.emit(max(-(-fq.steps // max(left, 1)) - 4, 0))
            fq.drain()
            cur_yts = [g.yts[(s % 2) * HP + fp] for fp in range(HP)]
            pending.append([
                (_proj_unit(g, s, c, out, cur_yts), 2 * HP) for c in range(NC_)
            ])
        for batch in pending:
            fq = _Fillers()
            for gen, steps in batch:
                fq.add(gen, steps)
            fq.drain()
    if split_waits:
        _split_waits(nc)
    return nc


_NC_CACHE = None


def _get_nc():
    global _NC_CACHE
    if _NC_CACHE is None:
        _NC_CACHE = build_nc()
    return _NC_CACHE


def make_in_maps(x, W_attn, b_attn, W_proj):
    x = np.asarray(x, dtype=np.float32)
    W_attn = np.asarray(W_attn, dtype=np.float32)
    b_attn = np.asarray(b_attn, dtype=np.float32)
    W_proj = np.asarray(W_proj, dtype=np.float32)
    bf = ml_dtypes.bfloat16
    in_maps = []
    for core in range(N_CORES):
        b, gg = core // 2, core % 2
        qs, ks, vs = gg * FC, C + gg * FC, 2 * C + gg * FC
        # x^T pre-tiled: [p, slab, ktile, t]
        xt = x[b].T.reshape(KT, 128, NSLAB, SLAB).transpose(1, 2, 0, 3)
        # weights pre-tiled: [p, ktile, f]
        wq = W_attn[:, qs : qs + FC].reshape(KT, 128, FC).transpose(1, 0, 2)
        wk = W_attn[:, ks : ks + FC].reshape(KT, 128, FC).transpose(1, 0, 2)
        wv = W_attn[:, vs : vs + FC].reshape(KT, 128, FC).transpose(1, 0, 2)
        wp = W_proj[gg * FC : (gg + 1) * FC, :].reshape(HP, 128, C).transpose(1, 0, 2)
        bqc = b_attn[qs : qs + FC].reshape(HP, 128).T
        bkc = b_attn[ks : ks + FC].reshape(HP, 128).T
        bias = np.stack([bqc, bqc + 1.0, bkc, bkc + 1.0], axis=1)
        in_maps.append({
            "xt": np.ascontiguousarray(xt).astype(bf),
            "wq": np.ascontiguousarray(wq).astype(bf),
            "wk": np.ascontiguousarray(wk).astype(bf),
            "wv": np.ascontiguousarray(wv).astype(bf),
            "wp": np.ascontiguousarray(wp).astype(bf),
            "bias": np.ascontiguousarray(bias.astype(np.float32)),
            "bvb": np.ascontiguousarray(
                np.broadcast_to(b_attn[vs : vs + FC][None, :], (128, FC)).astype(
                    np.float32
                )
            ),
        })
    return in_maps


def kernel(x, W_attn, b_attn, W_proj, b_proj, _trace=False, _tmpdir=None):
    nc = _get_nc()
    in_maps = make_in_maps(x, W_attn, b_attn, W_proj)
    try:
        res = run_bass_kernel_spmd(
            nc, in_maps, core_ids=list(range(N_CORES)), trace=_trace,
            tmpdir=_tmpdir,
        )
    except ModuleNotFoundError:
        # axon NTFF profiling hook unavailable in this environment
        res = run_bass_kernel_spmd(
            nc, in_maps, core_ids=list(range(N_CORES)), trace=False
        )
    b_proj = np.asarray(b_proj, dtype=np.float32)
    parts = [r["out"].astype(np.float32) for r in res.results]
    out = np.stack(
        [parts[2 * b] + parts[2 * b + 1] + b_proj for b in range(B)]
    ).astype(np.float32)
    kernel.last_results = res
    return out


# revision 39
# speedup vs baseline: 1.3300x; 1.3300x over previous
"""Trainium2 Bass kernel for linear causal self-attention (ELU+1 feature map).

Model (per batch b):
    qkv = x @ W_attn + b_attn ; q,k,v split; per-head (H=16, d=64)
    phi = elu(.)+1 applied to q,k
    causal linear attention: y_t = (phi_q_t . KV_t) / (phi_q_t . Ksum_t + eps)
        KV_t = sum_{s<=t} phi_k_s (x) v_s ; Ksum_t = sum_{s<=t} phi_k_s
    out = y @ W_proj + b_proj

Sharding (8 cores): core = 2*b + g  (b in 0..3 batches, g in 0..1 head-groups
of 8 heads).  Each core computes a partial output for its batch over its 8
heads; the host sums the two partials per batch and adds b_proj.

Performance design (from HW traces of earlier revisions):
  * The PE clock is HAM-throttled to 1.2 GHz whenever a ~3.4us activity
    window looks idle; small attention matmuls alone (65-128 cols) cannot
    keep it warm.  So the big 512-col qkv chains for slab s+1 and the
    output-projection chains for slab s-2 are emitted as *fine-grained
    fillers* - a few matmuls after every attention chunk unit - keeping the
    array duty-cycle high everywhere, including the last slab.
  * Attention chunks are S=128: no wasted half-block in the causal scores.
  * Per-head PSUM banks for scores/y: two accumulation groups from
    different PE row groups into one bank is a hard HW fault (bisected).
  * v is produced token-major directly (lhsT = x^T tile); y is normalized
    then PE-transposed per head-pair in ONE [128,128] transpose.
  * Host pre-lays-out all inputs so every DMA is 128 x 8KB descriptors;
    wq/xt0 are split across the two HWDGE rings so the first matmul can
    start ~14us in (3.3us init barrier + table loads are fixed runtime
    cost).  Output is written bf16 (tolerance 2e-2 >> bf16 rounding).
"""

from contextlib import ExitStack

import ml_dtypes
import numpy as np

import concourse.bass as bass
import concourse.mybir as mybir
import concourse.tile as tile
from concourse.bass_utils import run_bass_kernel_spmd
from concourse.masks import make_identity

F32 = mybir.dt.float32
BF16 = mybir.dt.bfloat16
AF = mybir.ActivationFunctionType

# Problem shape (hardcoded per harness contract)
B, T, C = 4, 2048, 1024
NH, D = 16, 64          # heads total, head dim
HG = 8                  # heads per core (head-group)
HP = 4                  # head-pairs per core (2 heads stacked on 128 partitions)
FC = HG * D             # 512 features per core per q/k/v
KT = C // 128           # 8 k-tiles of the model dim
SLAB = 512              # tokens per slab (qkv/proj granularity)
NSLAB = T // SLAB       # 4
NC_ = 4                 # chunks (128-token tiles) per slab
S128 = 128              # chunk length
N_CORES = 8


def _split_waits(nc, max_waits=1):
    """This walrus build accepts at most one embedded sync-wait per
    instruction; hoist extras into standalone EventSemaphore instructions."""
    n = 0
    for fn in nc.m.functions:
        for bb in fn.blocks:
            new_insts = []
            for inst in bb.instructions:
                si = inst.sync_info
                if si is not None and si.on_wait and len(si.on_wait) > max_waits:
                    extra = si.on_wait[:-max_waits]
                    keep = si.on_wait[-max_waits:]
                    for w in extra:
                        ev = mybir.InstEventSemaphore(
                            name=f"{inst.name}-wsplit{n}",
                            ins=[], outs=[],
                            engine=inst.engine,
                            sync_info=mybir.SyncInfo(on_wait=[w], on_update=[]),
                        )
                        n += 1
                        new_insts.append(ev)
                    si.on_wait = list(keep)
                new_insts.append(inst)
            bb.instructions = new_insts
    return n


class _Ctx:
    """Holds nc, pools and per-kernel constant tiles."""


def _setup_consts(g: _Ctx, dram):
    nc, consts = g.nc, g.consts
    # DMA ordering matters: each HWDGE ring completes in FIFO order.  The
    # first q-chain needs wq + xt0, so each is split half per ring and
    # leads its ring; everything else streams behind.
    g.wq_sb = consts.tile([128, KT, FC], BF16, tag="wq", name="wq")
    g.xt = []
    for s in range(NSLAB):
        g.xt.append(consts.tile([128, KT, SLAB], BF16, tag=f"xt{s}", name=f"xt{s}"))
    nc.sync.dma_start(g.wq_sb[:, 0 : KT // 2], dram["wq"][:, 0 : KT // 2])
    nc.scalar.dma_start(g.xt[0][:, 0 : KT // 2], dram["xt"][:, 0, 0 : KT // 2])
    nc.sync.dma_start(g.xt[0][:, KT // 2 :], dram["xt"][:, 0, KT // 2 :])
    nc.scalar.dma_start(g.wq_sb[:, KT // 2 :], dram["wq"][:, KT // 2 :])
    # per-partition phi biases: cols [bq, bq1, bk, bk1] x HP
    g.bias_sb = consts.tile([128, 4, HP], F32, tag="bias", name="bias")
    nc.scalar.dma_start(g.bias_sb, dram["bias"][:])
    g.wk_sb = consts.tile([128, KT, FC], BF16, tag="wk", name="wk")
    nc.sync.dma_start(g.wk_sb[:, 0 : KT // 2], dram["wk"][:, 0 : KT // 2])
    nc.scalar.dma_start(g.wk_sb[:, KT // 2 :], dram["wk"][:, KT // 2 :])
    nc.scalar.dma_start(g.xt[1], dram["xt"][:, 1])
    g.wv_sb = consts.tile([128, KT, FC], BF16, tag="wv", name="wv")
    nc.sync.dma_start(g.wv_sb, dram["wv"][:])
    # v-bias broadcast tile [128, FC]: every partition row = b_v
    g.bvb = consts.tile([128, FC], F32, tag="bvb", name="bvb")
    nc.scalar.dma_start(g.bvb, dram["bvb"][:])
    g.wp_sb = consts.tile([128, HP, C], BF16, tag="wp", name="wp")
    nc.sync.dma_start(g.wp_sb, dram["wp"][:])
    for s in range(2, NSLAB):
        nc.sync.dma_start(g.xt[s], dram["xt"][:, s])

    # causal mask for a [s_k(128), head(2), s_q(128)] score tile: keep
    # s_q >= s_k (same triangle in both head slots)
    g.mask = consts.tile([128, 2, S128], F32, tag="mask", name="mask")
    nc.vector.memset(g.mask, 1.0)
    for h in range(2):
        nc.gpsimd.affine_select(
            out=g.mask[:, h], in_=g.mask[:, h],
            compare_op=mybir.AluOpType.is_ge,
            fill=0.0, base=0, pattern=[[1, S128]], channel_multiplier=-1,
        )
    g.ident = consts.tile([128, 128], BF16, tag="ident", name="ident")
    make_identity(nc, g.ident)

    # combined state master (f32), all head-pairs: rows = 2 heads x 64,
    # per hp cols 0:64 = KV[i, j], col 64 = Ksum[i]
    g.kv = g.statep.tile([128, HP, D + 1], F32, tag="kv", name="kv")
    nc.vector.memset(g.kv, 0.0)


def _q_or_k_unit(g: _Ctx, s, hp, which):
    """One feature-major q/k chain + phi for slab s, head-pair hp.
    phi(y) = elu(y)+1 = max(min(exp(y), 1), y+1), computed as
    z = y+1 (ACT, bias), e = exp(y) (ACT, bias), e = min(e,1) (DVE),
    dst = max(z, e) (DVE).  Generator: yields after each PE instruction."""
    nc = g.nc
    wsb = g.wq_sb if which == "q" else g.wk_sb
    bcol = 0 if which == "q" else 2  # [bq, bq1, bk, bk1]
    ps = g.ps_big.tile([128, SLAB], F32, tag="big", name="qkps")
    xt_sb = g.xt[s]
    for k in range(KT):
        nc.tensor.matmul(
            ps,
            wsb[:, k, hp * 128 : (hp + 1) * 128],
            xt_sb[:, k, :],
            start=(k == 0),
            stop=(k == KT - 1),
        )
        if k < KT - 1:
            yield
    z = g.ep.tile([128, SLAB], BF16, tag="z", name="z")
    nc.scalar.activation(
        z, ps, AF.Identity, bias=g.bias_sb[:, bcol + 1, hp : hp + 1], scale=1.0
    )
    e = g.ep.tile([128, SLAB], BF16, tag="e", name="e")
    nc.scalar.activation(
        e, ps, AF.Exp, bias=g.bias_sb[:, bcol, hp : hp + 1], scale=1.0
    )
    nc.vector.tensor_scalar_min(e, e, 1.0)
    dst = g.qkp.tile([128, SLAB], BF16, tag=f"{which}t{hp}", name=f"{which}t{hp}")
    nc.vector.tensor_max(dst, z, e)
    lst = g.qt if which == "q" else g.kt_
    lst[(s % 2) * HP + hp] = dst
    yield


def _v_unit(g: _Ctx, s, tt):
    """Token-major v for slab s, token-tile tt: direct matmul with
    lhsT = x^T tile, plus broadcast bias add and the ones column."""
    nc = g.nc
    ps = g.ps_big.tile([128, SLAB], F32, tag="big", name="vps")
    xt_sb = g.xt[s]
    for k in range(KT):
        nc.tensor.matmul(
            ps[:, 0:FC],
            xt_sb[:, k, tt * 128 : (tt + 1) * 128],
            g.wv_sb[:, k, :],
            start=(k == 0),
            stop=(k == KT - 1),
        )
        if k < KT - 1:
            yield
    va = g.kvp.tile([128, HG, D + 1], BF16, tag=f"va{tt}", name=f"va{tt}")
    nc.vector.tensor_tensor(
        va[:, :, 0:D],
        ps[:, 0:FC].rearrange("p (a b) -> p a b", a=HG),
        g.bvb[:].rearrange("p (a b) -> p a b", a=HG),
        mybir.AluOpType.add,
    )
    nc.vector.memset(va[:, :, D], 1.0)
    g.vaug[(s % 2) * NC_ + tt] = va
    yield


def _tks_unit(g: _Ctx, s, tt):
    """PE-transpose phi(k) for token-tile tt into token-major kn."""
    nc = g.nc
    tks = g.ps_big.tile([128, HP, 128], BF16, tag="big", name="tks")
    for hp in range(HP):
        nc.tensor.transpose(
            tks[:, hp], g.kt_[(s % 2) * HP + hp][:, tt * 128 : (tt + 1) * 128],
            g.ident,
        )
        if hp < HP - 1:
            yield
    kn = g.kvp.tile([128, FC], BF16, tag=f"kn{tt}", name=f"kn{tt}")
    nc.scalar.activation(
        kn[:].rearrange("p (a b) -> p a b", a=HP), tks, AF.Copy
    )
    g.knat[(s % 2) * NC_ + tt] = kn
    yield


def _proj_unit(g: _Ctx, s, tt, out, yts_tiles):
    """Output projection for token-tile tt of slab s: two 512-wide chains
    into one [128, 1024] bf16 store.  yts_tiles are captured at creation
    time (the g.yts slots are rebound two slabs later)."""
    nc = g.nc
    t0 = s * SLAB + tt * 128
    osb = g.outp.tile([128, 2, FC], BF16, tag="osb", name="osb")
    for ec in range(2):
        ps = g.ps_big.tile([128, SLAB], F32, tag="big", name="projps")
        for fp in range(HP):
            nc.tensor.matmul(
                ps,
                yts_tiles[fp][:, tt * 128 : (tt + 1) * 128],
                g.wp_sb[:, fp, ec * FC : (ec + 1) * FC],
                start=(fp == 0),
                stop=(fp == HP - 1),
            )
            if fp < HP - 1 or ec == 0:
                yield
        nc.scalar.activation(osb[:, ec], ps, AF.Copy)
    nc.sync.dma_start(out[t0 : t0 + 128, :], osb[:].rearrange("p a b -> p (a b)"))
    yield


class _Fillers:
    """Queue of generator units; emit(n) advances across them n PE-steps."""

    def __init__(self):
        self.gens = []
        self.steps = 0  # total yields remaining (approximate bookkeeping)

    def add(self, gen, steps):
        self.gens.append(gen)
        self.steps += steps

    def emit(self, n):
        while n > 0 and self.gens:
            try:
                next(self.gens[0])
                self.steps -= 1
                n -= 1
            except StopIteration:
                self.gens.pop(0)

    def drain(self):
        while self.gens:
            self.emit(1 << 30)


def _chunk(g: _Ctx, s, c, hp, kvb, fq):
    """One (128-token chunk, head-pair) attention unit.  Filler matmuls are
    emitted at each cross-engine wait point: the PE executes in order, so a
    chunk matmul waiting on a DVE consumer would otherwise idle the array
    (and trip the HAM throttle) even with fillers queued after the chunk."""
    nc = g.nc
    q0 = c * S128
    si = s % 2
    qth = g.qt[si * HP + hp]
    kth = g.kt_[si * HP + hp]
    kn = g.knat[si * NC_ + c]
    va = g.vaug[si * NC_ + c]
    # scores P [s_k, s_q] per head, one 64-row matmul each (own bank:
    # two different-row-group groups in one bank is a HW fault)
    phs = []
    for h in range(2):
        r0, r1 = h * D, (h + 1) * D
        ph = getattr(g, f"ps_p{h}").tile([128, S128], F32, tag=f"p{h}", name=f"p{h}")
        nc.tensor.matmul(
            ph,
            kth[r0:r1, q0 : q0 + S128],
            qth[r0:r1, q0 : q0 + S128],
            tile_position=(h * D, 0),
        )
        phs.append(ph)
    pm = g.pmp.tile([128, 2, S128], BF16, tag="pm", name="pm")
    for h in range(2):
        nc.vector.tensor_mul(pm[:, h], phs[h], g.mask[:, h])
    # state update: KV += Knat^T @ V_aug (heads col-tiled, disjoint rows);
    # independent of the scores, so it runs while the mask-mul drains
    dlt = g.ps_d.tile([128, D + 1], F32, tag="d", name="dlt")
    for h in range(2):
        hv = hp * 2 + h
        nc.tensor.matmul(
            dlt[h * D : (h + 1) * D, :],
            kn[:, hv * D : (hv + 1) * D],
            va[:, hv],
            start=True, stop=True,
            tile_position=(0, h * D),
        )
    fq.emit(2)
    # y [s_q, (u, 65)] per head: intra (P @ V_aug) + inter (q . KV_aug).
    # Each matmul is split into token-halves on different PE column groups
    # (tile_position col 0/64): the halves run in separate quadrants
    # concurrently, and the 64-col stationaries load in half the time.
    ys = []
    for h in range(2):
        r0, r1 = h * D, (h + 1) * D
        hv = hp * 2 + h
        y = getattr(g, f"ps_y{h}").tile([128, D + 1], F32, tag=f"y{h}", name=f"y{h}")
        nc.tensor.matmul(y, pm[:, h], va[:, hv], start=True, stop=False)
        nc.tensor.matmul(
            y,
            qth[r0:r1, q0 : q0 + S128],
            kvb[r0:r1, hp, :],
            start=False, stop=True,
            tile_position=(h * D, 0),
        )
        ys.append(y)
    # normalize y_n = y[..., 0:64] / y[..., 64]; one [128,128] PE transpose
    # lands both heads directly in yts row order (h*64+d)
    yn = g.ynp.tile([128, 2, D], BF16, tag="yn", name="yn")
    for h in range(2):
        y = ys[h]
        rec = g.ynp.tile([128, 1], F32, tag="rec", name="rec")
        nc.vector.reciprocal(rec, y[:, D : D + 1])
        nc.vector.tensor_mul(yn[:, h], y[:, 0:D], rec[:].to_broadcast((128, D)))
    fq.emit(2)
    tp = g.ps_t.tile([128, 128], BF16, tag="tp", name="tp")
    nc.tensor.transpose(tp, yn[:].rearrange("p a b -> p (a b)"), g.ident)
    nc.scalar.activation(g.yts[si * HP + hp][:, q0 : q0 + S128], tp, AF.Copy)
    # state add (scheduled after the kvb copy for this row)
    nc.vector.tensor_add(g.kv[:, hp], g.kv[:, hp], dlt)


def _new_slab_tiles(g: _Ctx, s):
    si = s % 2
    for hp in range(HP):
        g.yts[si * HP + hp] = g.ytp.tile(
            [128, SLAB], BF16, tag=f"yt{hp}", name=f"yt{hp}"
        )


def build_nc(split_waits: bool = True) -> bass.Bass:
    nc = bass.Bass()
    dram = {
        "xt": nc.dram_tensor("xt", [128, NSLAB, KT, SLAB], BF16, kind="ExternalInput"),
        "wq": nc.dram_tensor("wq", [128, KT, FC], BF16, kind="ExternalInput"),
        "wk": nc.dram_tensor("wk", [128, KT, FC], BF16, kind="ExternalInput"),
        "wv": nc.dram_tensor("wv", [128, KT, FC], BF16, kind="ExternalInput"),
        "wp": nc.dram_tensor("wp", [128, HP, C], BF16, kind="ExternalInput"),
        "bias": nc.dram_tensor("bias", [128, 4, HP], F32, kind="ExternalInput"),
        "bvb": nc.dram_tensor("bvb", [128, FC], F32, kind="ExternalInput"),
    }
    out = nc.dram_tensor("out", [T, C], BF16, kind="ExternalOutput")

    with ExitStack() as ctx:
        tc = ctx.enter_context(tile.TileContext(nc))
        g = _Ctx()
        g.nc = nc
        for nm, kw in (
            ("consts", dict(bufs=1)),
            ("qkp", dict(bufs=2)),
            ("kvp", dict(bufs=2)),
            ("ytp", dict(bufs=3)),
            ("ep", dict(bufs=4)),
            ("pmp", dict(bufs=3)),
            ("ynp", dict(bufs=4)),
            ("outp", dict(bufs=3)),
            ("statep", dict(bufs=1)),
            ("kvbp", dict(bufs=2)),
            ("ps_big", dict(bufs=2, space="PSUM")),
            ("ps_p0", dict(bufs=1, space="PSUM")),
            ("ps_p1", dict(bufs=1, space="PSUM")),
            ("ps_y0", dict(bufs=1, space="PSUM")),
            ("ps_y1", dict(bufs=1, space="PSUM")),
            ("ps_d", dict(bufs=1, space="PSUM")),
            ("ps_t", dict(bufs=1, space="PSUM")),
        ):
            setattr(g, nm, ctx.enter_context(tc.tile_pool(name=nm, **kw)))

        # slots for double-buffered per-slab tensors (slab s uses s % 2)
        g.qt = [None] * (2 * HP)
        g.kt_ = [None] * (2 * HP)
        g.knat = [None] * (2 * NC_)
        g.vaug = [None] * (2 * NC_)
        g.yts = [None] * (2 * HP)

        _setup_consts(g, dram)

        def add_qkvnat(fq, s):
            for hp in range(HP):
                fq.add(_q_or_k_unit(g, s, hp, "q"), KT)
            for hp in range(HP):
                fq.add(_q_or_k_unit(g, s, hp, "k"), KT)
            for tt in range(NC_):
                fq.add(_tks_unit(g, s, tt), HP)
            for tt in range(NC_):
                fq.add(_v_unit(g, s, tt), KT)

        # prologue: slab 0 q/k chains + first token-tile naturalize, dense.
        # tks/v for tiles 1-3 are held back as extra filler for slab 0's
        # attention (the thinnest filler zone: it has only qkv(1) otherwise).
        fq0 = _Fillers()
        for hp in range(HP):
            fq0.add(_q_or_k_unit(g, 0, hp, "q"), KT)
        for hp in range(HP):
            fq0.add(_q_or_k_unit(g, 0, hp, "k"), KT)
        fq0.add(_tks_unit(g, 0, 0), HP)
        fq0.add(_v_unit(g, 0, 0), KT)
        fq0.drain()
        held = []
        for tt in range(1, NC_):
            held.append((_tks_unit(g, 0, tt), HP))
            held.append((_v_unit(g, 0, tt), KT))

        # proj units for slab s are deferred TWO slabs: they are the PE
        # filler through slab s+2's attention (keeps the tail warm).
        pending = [[], []]
        for s in range(NSLAB):
            _new_slab_tiles(g, s)
            fq = _Fillers()
            if s == 0:
                for gen, steps in held:
                    fq.add(gen, steps)
            if s + 1 < NSLAB:
                add_qkvnat(fq, s + 1)
            for gen, steps in pending.pop(0):
                fq.add(gen, steps)
            nchunks = NC_ * HP
            for c in range(NC_):
                # held naturalize units for this row must have been emitted
                while g.knat[(s % 2) * NC_ + c] is None or (
                    g.vaug[(s % 2) * NC_ + c] is None
                ):
                    fq.emit(1)
                kvb = g.kvbp.tile([128, HP, D + 1], BF16, tag="kvb", name="kvb")
                nc.scalar.activation(kvb, g.kv, AF.Copy)
                for hp in range(HP):
                    _chunk(g, s, c, hp, kvb, fq)
                    left = NC_ * HP - (c * HP + hp)
                    fq.emit(max(-(-fq.steps // max(left, 1)) - 4, 0))
            fq.drain()
            cur_yts = [g.yts[(s % 2) * HP + fp] for fp in range(HP)]
            pending.append([
                (_proj_unit(g, s, c, out, cur_yts), 2 * HP) for c in range(NC_)
            ])
        for batch in pending:
            fq = _Fillers()
            for gen, steps in batch:
                fq.add(gen, steps)
            fq.drain()
    if split_waits:
        _split_waits(nc)
    return nc


_NC_CACHE = None


def _get_nc():
    global _NC_CACHE
    if _NC_CACHE is None:
        _NC_CACHE = build_nc()
    return _NC_CACHE


def make_in_maps(x, W_attn, b_attn, W_proj):
    x = np.asarray(x, dtype=np.float32)
    W_attn = np.asarray(W_attn, dtype=np.float32)
    b_attn = np.asarray(b_attn, dtype=np.float32)
    W_proj = np.asarray(W_proj, dtype=np.float32)
    bf = ml_dtypes.bfloat16
    in_maps = []
    for core in range(N_CORES):
        b, gg = core // 2, core % 2
        qs, ks, vs = gg * FC, C + gg * FC, 2 * C + gg * FC
        # x^T pre-tiled: [p, slab, ktile, t]
        xt = x[b].T.reshape(KT, 128, NSLAB, SLAB).transpose(1, 2, 0, 3)
        # weights pre-tiled: [p, ktile, f]
        wq = W_attn[:, qs : qs + FC].reshape(KT, 128, FC).transpose(1, 0, 2)
        wk = W_attn[:, ks : ks + FC].reshape(KT, 128, FC).transpose(1, 0, 2)
        wv = W_attn[:, vs : vs + FC].reshape(KT, 128, FC).transpose(1, 0, 2)
        wp = W_proj[gg * FC : (gg + 1) * FC, :].reshape(HP, 128, C).transpose(1, 0, 2)
        bqc = b_attn[qs : qs + FC].reshape(HP, 128).T
        bkc = b_attn[ks : ks + FC].reshape(HP, 128).T
        bias = np.stack([bqc, bqc + 1.0, bkc, bkc + 1.0], axis=1)
        in_maps.append({
            "xt": np.ascontiguousarray(xt).astype(bf),
            "wq": np.ascontiguousarray(wq).astype(bf),
            "wk": np.ascontiguousarray(wk).astype(bf),
            "wv": np.ascontiguousarray(wv).astype(bf),
            "wp": np.ascontiguousarray(wp).astype(bf),
            "bias": np.ascontiguousarray(bias.astype(np.float32)),
            "bvb": np.ascontiguousarray(
                np.broadcast_to(b_attn[vs : vs + FC][None, :], (128, FC)).astype(
                    np.float32
                )
            ),
        })
    return in_maps


def kernel(x, W_attn, b_attn, W_proj, b_proj, _trace=False, _tmpdir=None):
    nc = _get_nc()
    in_maps = make_in_maps(x, W_attn, b_attn, W_proj)
    try:
        res = run_bass_kernel_spmd(
            nc, in_maps, core_ids=list(range(N_CORES)), trace=_trace,
            tmpdir=_tmpdir,
        )
    except ModuleNotFoundError:
        # axon NTFF profiling hook unavailable in this environment
        res = run_bass_kernel_spmd(
            nc, in_maps, core_ids=list(range(N_CORES)), trace=False
        )
    b_proj = np.asarray(b_proj, dtype=np.float32)
    parts = [r["out"].astype(np.float32) for r in res.results]
    out = np.stack(
        [parts[2 * b] + parts[2 * b + 1] + b_proj for b in range(B)]
    ).astype(np.float32)
    kernel.last_results = res
    return out
